# revision 1
# baseline (speedup 1.0000x reference)
"""Multi-head attention (12 heads, RoPE, causal SDPA) for Trainium2, 8 cores.

Sharding: batch (2) x head-group (4 groups of 3 heads). Each core computes,
for its (batch b, head-group hg): QKV projection for its 3 heads, RoPE,
causal attention, and a partial out-projection [T, C] restricted to its
heads' rows of w_out. The host sums the 4 head-group partials per batch.

Device-side layouts (T = 2048, C = 768, D = 64 per head):
  xT   [768, 2048]  x[b] transposed (c on partitions)
  wA   [768, 640]   packed lhsT weights: cols 0:128 [q0|q1], 128:256 [k0|k1],
                    256:320 q2, 320:384 k2, 384:576 w_v (3 heads), 64 zero pad
  wo   [64, 2304]   w_out rows for this head-group: 3 x [64 d, 768 c]
  cosT/sinT [128, 2048]  RoPE tables transposed, stacked twice (64 d x 2)
  rT   [128, 128]   rotate_half as matmul lhsT: rot(q)T_chunk = rT.T @ qT_chunk
  tri  [128, 128]   tri[kr, qc] = 1 if qc >= kr (causal keep-mask, S^T layout)

Attention is computed transposed (S^T[k, q] = K Q^T blocks) so that softmax
P^T lands in [k, q] layout, which feeds P@V directly with v in natural [t, d]
layout (no transposes). Softmax has no max-subtraction (scores are O(1) by
construction) and the denominator comes from an all-ones column appended to
the stationary v operand. Normalization is applied in the [d, q] layout via a
K=1 outer-product broadcast of 1/denominator. Matmuls use float32r (~13
mantissa bits, 4x faster than fp32 on the PE).
"""
import numpy as np

B, T, C, H, D = 2, 2048, 768, 12, 64
HPG = 3                    # heads per group
NG = B * (H // HPG)        # 8 cores
ROPE_BASE = 10000.0
TQ = T // 128              # 16 t-tiles
NCC = C // 128             # 6 contraction chunks
GW = 1024                  # attention q-group width
NGRP = T // GW             # 2 q-groups

_CACHE = {}


def _build_nc(reps=1):
    from concourse import bacc, tile, mybir

    f32 = mybir.dt.float32
    f32r = mybir.dt.float32r
    Exp = mybir.ActivationFunctionType.Exp
    mult = mybir.AluOpType.mult
    add = mybir.AluOpType.add

    nc = bacc.Bacc("TRN2", target_bir_lowering=False, debug=False,
                   num_devices=NG)

    xT_d = nc.dram_tensor("xT", [C, T], f32r, kind="ExternalInput").ap()
    wA_d = nc.dram_tensor("wA", [C, 640], f32r, kind="ExternalInput").ap()
    woA_d = nc.dram_tensor("woA", [2 * D, C], f32r, kind="ExternalInput").ap()
    woB_d = nc.dram_tensor("woB", [D, C], f32r, kind="ExternalInput").ap()
    cosT_d = nc.dram_tensor("cosT", [128, T], f32, kind="ExternalInput").ap()
    sinT_d = nc.dram_tensor("sinT", [128, T], f32, kind="ExternalInput").ap()
    rT_d = nc.dram_tensor("rT", [128, 128], f32r, kind="ExternalInput").ap()
    tri_d = nc.dram_tensor("tri", [128, 128], f32, kind="ExternalInput").ap()
    out_d = nc.dram_tensor("out", [T, C], f32, kind="ExternalOutput").ap()

    with tile.TileContext(nc) as tc:
      for rep in range(reps):
        with tc.tile_pool(name=f"persist{rep}", bufs=1) as pp:
                dmaq = [nc.sync, nc.scalar, nc.gpsimd]

                # ---- persistent constants ----
                woA = pp.tile([2 * D, C], f32r, tag="woA")
                dmaq[1].dma_start(woA[:], woA_d[:])
                woB = pp.tile([D, C], f32r, tag="woB")
                dmaq[1].dma_start(woB[:], woB_d[:])
                tri = pp.tile([128, 128], f32, tag="tri")
                dmaq[2].dma_start(tri[:], tri_d[:])
                onesf = pp.tile([1, D], f32, tag="onesf")
                nc.vector.memset(onesf[:], 1.0)
                ones = pp.tile([1, D], f32r, tag="ones")
                nc.scalar.copy(ones[:], onesf[:])

                # persistent intermediates: [q0|q1], [k0|k1], [q2], [k2]
                # (projection computes [q2|k2] packed; RoPE splits into two
                # 64-row tiles via cross-partition DVE writes)
                qk_rows = [128, 128, 64, 64]
                qkT = [pp.tile([qk_rows[m], T], f32r, tag=f"qkT{m}",
                               name=f"qkT{m}") for m in range(4)]
                v_sb = pp.tile([128, TQ, HPG, 65], f32r, tag="v_sb")
                onesw = pp.tile([128, TQ * HPG], f32, tag="onesw")
                nc.vector.memset(onesw[:], 1.0)
                nc.scalar.copy(
                    v_sb[:, :, :, 64:65],
                    onesw[:].rearrange("p (a b) -> p a b", b=HPG).rearrange(
                        "p a b -> p a b ()"))
                attnT_A = pp.tile([2 * D, T], f32r, tag="attnTA")
                attnT_B = pp.tile([D, T], f32r, tag="attnTB")
                attn_dst = [(attnT_A, 0), (attnT_A, D), (attnT_B, 0)]

                # ================= QKV phase (scoped pools) =================
                qkv_pool = tc.tile_pool(name=f"qkv{rep}", bufs=1)
                qp = qkv_pool.__enter__()
                qkv_ps_pool = tc.tile_pool(name=f"qkvps{rep}", bufs=8, space="PSUM")
                qps = qkv_ps_pool.__enter__()

                xT = [qp.tile([128, T], f32r, tag=f"xT{c}", name=f"xT{c}")
                      for c in range(NCC)]
                wA = [qp.tile([128, 640], f32r, tag=f"wA{c}", name=f"wA{c}")
                      for c in range(NCC)]
                # weights first (small), then xT column-major in [128, 512]
                # pieces so the first projection chunk's deps arrive in ~2us
                for c in range(NCC):
                    dmaq[c % 3].dma_start(
                        wA[c][:], wA_d[128 * c:128 * (c + 1), :])
                qi = 0
                for n in range(4):
                    for c in range(NCC):
                        nsl = slice(512 * n, 512 * (n + 1))
                        dmaq[qi % 3].dma_start(
                            xT[c][:, nsl], xT_d[128 * c:128 * (c + 1), nsl])
                        qi += 1
                cosT = qp.tile([128, T], f32, tag="cosT")
                sinT = qp.tile([128, T], f32, tag="sinT")
                dmaq[2].dma_start(cosT[:], cosT_d[:])
                dmaq[0].dma_start(sinT[:], sinT_d[:])
                rT = qp.tile([128, 128], f32r, tag="rT")
                dmaq[1].dma_start(rT[:], rT_d[:])

                # q/k projection + RoPE; rot matmuls lag the raw projections
                # by two chunks so PE never stalls on the ACT psum->sbuf copy
                qk_cols = [(0, 128), (128, 256), (256, 384)]
                chunks = [(m, n) for n in range(4) for m in range(3)]
                raws = {}

                def emit_raw(i):
                    m, n = chunks[i]
                    c0, c1 = qk_cols[m]
                    rows = 128
                    tsl = slice(512 * n, 512 * (n + 1))
                    praw = qps.tile([128, 512], f32, tag="ps", name=f"praw{i}")
                    for c in range(NCC):
                        nc.tensor.matmul(
                            praw[0:rows, :], wA[c][:, c0:c1], xT[c][:, tsl],
                            start=(c == 0), stop=(c == NCC - 1))
                    raw = qp.tile([128, 512], f32r, tag="raw", bufs=5,
                                  name=f"raw{i}")
                    nc.scalar.copy(raw[0:rows, :], praw[0:rows, :])
                    raws[i] = raw

                def emit_rope(i):
                    m, n = chunks[i]
                    tsl = slice(512 * n, 512 * (n + 1))
                    raw = raws.pop(i)
                    prot = qps.tile([128, 512], f32, tag="ps", name=f"prot{i}")
                    nc.tensor.matmul(prot[:], rT[:], raw[:], start=True,
                                     stop=True)
                    t1 = qp.tile([128, 512], f32, tag="t1", bufs=3,
                                 name=f"t1_{i}")
                    nc.gpsimd.tensor_tensor(t1[:], raw[:], cosT[:, tsl], mult)
                    t2 = qp.tile([128, 512], f32, tag="t2", bufs=3,
                                 name=f"t2_{i}")
                    nc.vector.tensor_tensor(t2[:], prot[:], sinT[:, tsl], mult)
                    if m < 2:
                        nc.vector.tensor_tensor(qkT[m][:, tsl], t1[:], t2[:],
                                                add)
                    else:
                        # packed [q2|k2]: split to qkT[2]/qkT[3] (cross-part)
                        nc.vector.tensor_tensor(qkT[2][:, tsl], t1[0:64, :],
                                                t2[0:64, :], add)
                        nc.vector.tensor_tensor(qkT[3][:, tsl], t1[64:128, :],
                                                t2[64:128, :], add)

                for i in range(len(chunks)):
                    emit_raw(i)
                    if i >= 2:
                        emit_rope(i - 2)
                for i in (len(chunks) - 2, len(chunks) - 1):
                    emit_rope(i)

                # V projection in natural [t, d] layout
                for t in range(TQ):
                    tsl = slice(128 * t, 128 * (t + 1))
                    pv = qps.tile([128, 256], f32, tag="ps", name=f"pv{t}")
                    for c in range(NCC):
                        nc.tensor.matmul(pv[:], xT[c][:, tsl],
                                         wA[c][:, 384:640], start=(c == 0),
                                         stop=(c == NCC - 1))
                    nc.vector.tensor_copy(
                        v_sb[:, t, :, 0:64],
                        pv[:, 0:192].rearrange("p (h d) -> p h d", d=64))

                qkv_ps_pool.__exit__(None, None, None)
                qkv_pool.__exit__(None, None, None)

                # ========== attention + out projection (interleaved) ==========
                attn_pool = tc.tile_pool(name=f"attn{rep}", bufs=1)
                ap = attn_pool.__enter__()
                attn_ps_pool = tc.tile_pool(name=f"attnps{rep}", bufs=2, space="PSUM")
                aps = attn_ps_pool.__enter__()

                # q/k row views per head: (tile index, partition offset)
                qv = [(0, 0), (0, 64), (2, 0)]
                kv = [(1, 0), (1, 64), (3, 0)]

                for g in range(NGRP):
                    for h in range(HPG):
                        qm, qo = qv[h]
                        km, ko = kv[h]
                        qT = qkT[qm][qo:qo + 64, :]
                        kT = qkT[km][ko:ko + 64, :]
                        nj = (GW // 128) * (g + 1)
                        # pass A: scores + exp (+ causal tri) for every k-chunk
                        pts = []
                        for j in range(nj):
                            dj = j - (GW // 128) * g
                            col0 = 128 * dj if dj >= 0 else 0
                            pscr = aps.tile([128, GW], f32, tag="pscr", bufs=2,
                                            name=f"pscr{g}_{h}_{j}")
                            for s0 in range(col0 - col0 % 512, GW, 512):
                                a0 = max(s0, col0)
                                nc.tensor.matmul(
                                    pscr[:, a0:s0 + 512],
                                    kT[:, 128 * j:128 * (j + 1)],
                                    qT[:, GW * g + a0:GW * g + s0 + 512],
                                    start=True, stop=True)
                            pt = ap.tile([128, GW], f32r, tag="pt", bufs=17,
                                         name=f"pt{g}_{h}_{j}")
                            nc.scalar.activation(pt[:, col0:], pscr[:, col0:],
                                                 Exp, scale=0.125)
                            if dj >= 0:
                                nc.gpsimd.tensor_tensor(
                                    pt[:, col0:col0 + 128],
                                    pt[:, col0:col0 + 128], tri[:], mult)
                            pts.append((pt, col0))
                        # pass B: P^T @ V into two 512-wide accumulators
                        pos = [aps.tile([65, 512], f32, tag="pso", bufs=4,
                                        name=f"po{g}_{h}_{i2}")
                               for i2 in range(GW // 512)]
                        lastw = {}
                        for j in range(nj):
                            _, col0 = pts[j]
                            for s0 in range(col0 - col0 % 512, GW, 512):
                                lastw[s0 // 512] = j
                        for j in range(nj):
                            pt, col0 = pts[j]
                            for s0 in range(col0 - col0 % 512, GW, 512):
                                a0 = max(s0, col0)
                                hv = s0 // 512
                                nc.tensor.matmul(
                                    pos[hv][:, a0 - s0:512], v_sb[:, j, h, :],
                                    pt[:, a0:s0 + 512], start=(j == 0),
                                    stop=(j == lastw[hv]), skip_group_check=True)
                        # normalize per half: attnT = po[0:64] * (1/po[64]),
                        # denominator broadcast across partitions on GPSIMD
                        for hv in range(GW // 512):
                            po = pos[hv]
                            csl = slice(GW * g + 512 * hv, GW * g + 512 * (hv + 1))
                            rc0 = ap.tile([1, 512], f32, tag="rc0", bufs=2,
                                          name=f"rc0{g}_{h}_{hv}")
                            nc.vector.reciprocal(rc0[:], po[64:65, :])
                            pbb = ap.tile([64, 512], f32, tag="pbb", bufs=3,
                                          name=f"pbb{g}_{h}_{hv}")
                            nc.gpsimd.partition_broadcast(pbb[:], rc0[:])
                            dstT, dofs = attn_dst[h]
                            nc.vector.tensor_tensor(dstT[dofs:dofs + D, csl],
                                                    po[0:64, :], pbb[:], mult)

                    # out projection for this g's t-range, from the same pool
                    for t in range((TQ // NGRP) * g, (TQ // NGRP) * (g + 1)):
                        tsl = slice(128 * t, 128 * (t + 1))
                        for c0, cn in ((0, 512), (512, 256)):
                            pout = aps.tile([128, cn], f32, tag="pso", bufs=4,
                                            name=f"pout{t}_{c0}")
                            nc.tensor.matmul(pout[:], attnT_A[:, tsl],
                                             woA[:, c0:c0 + cn], start=True,
                                             stop=False)
                            nc.tensor.matmul(pout[:], attnT_B[:, tsl],
                                             woB[:, c0:c0 + cn], start=False,
                                             stop=True)
                            osb = ap.tile([128, cn], f32, tag=f"osb{c0}", bufs=3,
                                          name=f"osb{t}_{c0}")
                            nc.any.tensor_copy(osb[:], pout[:])
                            dmaq[2 * ((t + (1 if c0 else 0)) % 2)].dma_start(
                                out_d[tsl, c0:c0 + cn], osb[:])

                attn_ps_pool.__exit__(None, None, None)
                attn_pool.__exit__(None, None, None)

    nc.compile()
    return nc


def _host_inputs(x, w_qkv, w_out):
    """Build the 8 per-core input maps."""
    inv_freq = 1.0 / (ROPE_BASE ** (np.arange(0, D, 2, dtype=np.float32) / D))
    t = np.arange(T, dtype=np.float32)
    freqs = t[:, None] * inv_freq[None, :]          # [T, D/2]
    emb = np.concatenate([freqs, freqs], axis=-1)   # [T, D]
    cosT = np.ascontiguousarray(np.cos(emb).T.astype(np.float32))  # [D, T]
    sinT = np.ascontiguousarray(np.sin(emb).T.astype(np.float32))
    cosT2 = np.concatenate([cosT, cosT], axis=0)    # [128, T]
    sinT2 = np.concatenate([sinT, sinT], axis=0)

    # rotate_half permutation as matmul lhsT: rot = R @ q, lhsT = R.T
    R = np.zeros((D, D), np.float32)
    R[0:32, 32:64] = -np.eye(32)
    R[32:64, 0:32] = np.eye(32)
    R2 = np.zeros((128, 128), np.float32)
    R2[0:64, 0:64] = R
    R2[64:128, 64:128] = R
    rT = np.ascontiguousarray(R2.T)

    tri = np.zeros((128, 128), np.float32)
    for kr in range(128):
        tri[kr, kr:] = 1.0

    wq = w_qkv[0:C]
    wk = w_qkv[C:2 * C]
    wv = w_qkv[2 * C:3 * C]

    maps = []
    for core in range(NG):
        b, hg = core // 4, core % 4
        hs = slice(HPG * D * hg, HPG * D * (hg + 1))   # 192 rows of this group
        h2 = HPG * D * hg + 2 * D
        q01 = wq[hs][0:128]                             # [128, C]
        k01 = wk[hs][0:128]
        q2 = wq[h2:h2 + D]
        k2 = wk[h2:h2 + D]
        v3 = wv[hs]                                     # [192, C]
        wA = np.zeros((C, 640), np.float32)
        wA[:, 0:128] = q01.T
        wA[:, 128:256] = k01.T
        wA[:, 256:320] = q2.T
        wA[:, 320:384] = k2.T
        wA[:, 384:576] = v3.T
        wo_h = [w_out[:, HPG * D * hg + D * h: HPG * D * hg + D * (h + 1)].T
                for h in range(HPG)]                    # 3 x [64, C]
        woA = np.concatenate([wo_h[0], wo_h[1]], axis=0)  # [128, C]
        woB = wo_h[2]                                     # [64, C]
        maps.append({
            "xT": np.ascontiguousarray(x[b].T),
            "wA": np.ascontiguousarray(wA),
            "woA": np.ascontiguousarray(woA.astype(np.float32)),
            "woB": np.ascontiguousarray(woB.astype(np.float32)),
            "cosT": cosT2, "sinT": sinT2,
            "rT": rT, "tri": tri,
        })
    return maps


def kernel(x, w_qkv, w_out):
    from concourse.bass_utils import run_bass_kernel_spmd

    if "nc" not in _CACHE:
        _CACHE["nc"] = _build_nc()
    nc = _CACHE["nc"]

    maps = _host_inputs(np.asarray(x, np.float32),
                        np.asarray(w_qkv, np.float32),
                        np.asarray(w_out, np.float32))
    res = run_bass_kernel_spmd(nc, maps, core_ids=list(range(NG))).results
    parts = np.stack([r["out"] for r in res])           # [8, T, C]
    out = np.zeros((B, T, C), np.float32)
    for b in range(B):
        out[b] = parts[4 * b:4 * (b + 1)].sum(axis=0)
    return out



# revision 13
# speedup vs baseline: 1.1819x; 1.1819x over previous
"""Multi-head attention (12 heads, RoPE, causal SDPA) for Trainium2, 8 cores.

Sharding: batch (2) x head-group (4 groups of 3 heads). Each core computes,
for its (batch b, head-group hg): QKV projection for its 3 heads, RoPE,
causal attention, and a partial out-projection [T, C] restricted to its
heads' rows of w_out. The host sums the 4 head-group partials per batch.

v3 design notes (CoreSim v1 cost-model driven):
  - Matmul cost = out_free_size * pe_cycle * cycles_per_row, independent of
    contraction depth; stationary (lhsT) loads are free. fp8e4 DoubleRow is
    0.5 cycles/row and contracts two 128-chunks per instruction.
  - fp8 (e4m3, ~3% rms) is only tolerable on the q/k logits path; every
    value-path tensor (v, p, attn, w_out) needs bf16-or-better. Hence:
      q/k proj:  single-fp8 DoubleRow (x_hi (x) w),      9216 PE cycles
      v proj:    3-term hi+lo fp8 DoubleRow (exact-ish), 13824
      scores:    bf16, transposed S^T[k,q] layout,       52224
      P@V:       bf16 NATURAL orientation - stationary pt chunk [128k,128q],
                 moving v [128k, 65] -> cost 65/matmul,  26520
      out proj:  bf16 (attnT stationary),                24576
      rope rot:  bf16 matmul,                            6144
  - exp() is Activation-engine-only and the global bottleneck (~52k psum
    cols * 0.83ns + ~185ns/instr ~= 57us); ACT does nothing else.
  - P@V natural output [128q, 4*65] accumulates in psum with the ones*64
    column giving the denominator; normalize = one Pool tensor_scalar
    divide per q-block (per-partition scalar); attn^T for the out-proj
    comes from dma_start_transpose (14ns/32x32 tile, on the SP queue).
  - q01/k01 projected first so heads 0/1 start attention ~3us in; V + q2k2
    run under the early exp stream. PSUM: qkv ring 2 banks + pscr 2x2 +
    po ring 2; the outproj ring reuses the qkv banks after it closes.

Device layouts (T=2048, C=768, D=64): see _host_inputs.
"""
import numpy as np

B, T, C, H, D = 2, 2048, 768, 12, 64
HPG = 3                    # heads per group
NG = B * (H // HPG)        # 8 cores
ROPE_BASE = 10000.0
TQ = T // 128              # 16 t-tiles
GW = 1024                  # attention q-group width
NGRP = T // GW             # 2 q-groups
SW = 64.0                  # fp8 weight prescale (q/k/v carry *64)
SC = 0.125 / (SW * SW)     # exp scale

_CACHE = {}


def _build_nc(reps=1):
    from concourse import bacc, tile, mybir

    f32 = mybir.dt.float32
    bf16 = mybir.dt.bfloat16
    fp8 = mybir.dt.float8e4
    Exp = mybir.ActivationFunctionType.Exp
    mult = mybir.AluOpType.mult
    add = mybir.AluOpType.add
    divide = mybir.AluOpType.divide
    DR = mybir.MatmulPerfMode.DoubleRow

    nc = bacc.Bacc("TRN2", target_bir_lowering=False, debug=False,
                   num_devices=NG)

    xh_d = nc.dram_tensor("xh8", [128, 6, T], fp8, kind="ExternalInput").ap()
    xl_d = nc.dram_tensor("xl8", [128, 6, T], fp8, kind="ExternalInput").ap()
    # wA8[p, pair, half, col]: cols = [q01(128) | k01(128) | q2|k2(128) | v(192)]
    wA_d = nc.dram_tensor("wA8", [128, 3, 2, 576], fp8,
                          kind="ExternalInput").ap()
    wAl_d = nc.dram_tensor("wAl8", [128, 3, 2, 192], fp8,
                           kind="ExternalInput").ap()
    woA_d = nc.dram_tensor("woA", [128, C], bf16, kind="ExternalInput").ap()
    woB_d = nc.dram_tensor("woB", [64, C], bf16, kind="ExternalInput").ap()
    cosT_d = nc.dram_tensor("cosT", [128, T], bf16, kind="ExternalInput").ap()
    sinT_d = nc.dram_tensor("sinT", [128, T], bf16, kind="ExternalInput").ap()
    rT_d = nc.dram_tensor("rT", [128, 128], bf16, kind="ExternalInput").ap()
    mA_d = nc.dram_tensor("maskA", [128, 128], bf16, kind="ExternalInput").ap()
    out_d = nc.dram_tensor("out", [T, C], bf16, kind="ExternalOutput").ap()

    with tile.TileContext(nc) as tc:
      for rep in range(reps):
        with tc.tile_pool(name=f"persist{rep}", bufs=1) as pp:
                # DMA queues: SP-heavy, Pool secondary; ACT only does exp.
                dq = [nc.sync, nc.gpsimd]

                # ---- persistent constants ----
                wA = pp.tile([128, 3, 2, 576], fp8, tag="wA")
                dq[0].dma_start(wA[:], wA_d[:])
                wAl = pp.tile([128, 3, 2, 192], fp8, tag="wAl")
                dq[1].dma_start(wAl[:], wAl_d[:])
                woA = pp.tile([128, C], bf16, tag="woA")
                dq[1].dma_start(woA[:], woA_d[:])
                woB = pp.tile([64, C], bf16, tag="woB")
                dq[1].dma_start(woB[:], woB_d[:])
                rT = pp.tile([128, 128], bf16, tag="rT")
                dq[1].dma_start(rT[:], rT_d[:])
                maskA = pp.tile([128, 128], bf16, tag="maskA")
                dq[0].dma_start(maskA[:], mA_d[:])
                cosT = pp.tile([128, T], bf16, tag="cosT")
                sinT = pp.tile([128, T], bf16, tag="sinT")
                for hlf in range(2):
                    hsl = slice(1024 * hlf, 1024 * (hlf + 1))
                    dq[hlf].dma_start(cosT[:, hsl], cosT_d[:, hsl])
                    dq[1 - hlf].dma_start(sinT[:, hsl], sinT_d[:, hsl])
                xh = pp.tile([128, 6, T], fp8, tag="xh")
                xl = pp.tile([128, 6, T], fp8, tag="xl")
                for n in range(4):
                    nsl = slice(512 * n, 512 * (n + 1))
                    dq[n % 2].dma_start(xh[:, :, nsl], xh_d[:, :, nsl])
                    dq[(n + 1) % 2].dma_start(xl[:, :, nsl], xl_d[:, :, nsl])

                v_sb = pp.tile([128, TQ, HPG, 65], bf16, tag="v_sb")
                nc.vector.memset(v_sb[:, :, :, 64:65], SW)

                # q/k rope outputs (bf16): [q0|q1], [k0|k1], q2, k2
                q01 = pp.tile([128, T], bf16, tag="q01")
                k01 = pp.tile([128, T], bf16, tag="k01")
                q2 = pp.tile([64, T], bf16, tag="q2")
                k2 = pp.tile([64, T], bf16, tag="k2")
                attnT_A = pp.tile([128, T], bf16, tag="attnTA")
                attnT_B = pp.tile([128, T], bf16, tag="attnTB")

                # ---- psum pools ----
                attn_ps_pool = tc.tile_pool(name=f"attnps{rep}", bufs=2,
                                            space="PSUM")
                aps = attn_ps_pool.__enter__()     # pscr [128,1024] x2 = 4 banks
                po_ps_pool = tc.tile_pool(name=f"pops{rep}", bufs=2,
                                          space="PSUM")
                ops = po_ps_pool.__enter__()       # po [128,4,65] x2 = 2 banks
                qkv_ps_pool = tc.tile_pool(name=f"qkvps{rep}", bufs=2,
                                           space="PSUM")
                qps = qkv_ps_pool.__enter__()      # [128,512] x2 = 2 banks

                # ================= QKV phase =================
                qk_cols = [(0, 128), (128, 256), (256, 384)]
                chunks = [(m, n) for n in range(4) for m in range(2)]
                chunks += [(2, n) for n in range(4)]
                raws = {}

                def emit_raw(i):
                    m, n = chunks[i]
                    c0, c1 = qk_cols[m]
                    tsl = slice(512 * n, 512 * (n + 1))
                    praw = qps.tile([128, 512], f32, tag="qps",
                                    name=f"praw{i}")
                    for p in range(3):
                        nc.tensor.matmul(
                            praw[:], wA[:, p, :, c0:c1],
                            xh[:, 2 * p:2 * p + 2, tsl],
                            start=(p == 0), stop=(p == 2), perf_mode=DR)
                    raw = pp.tile([128, 512], bf16, tag="raw", bufs=5,
                                  name=f"raw{i}")
                    nc.gpsimd.tensor_copy(raw[:], praw[:])
                    raws[i] = raw

                def emit_rope(i):
                    m, n = chunks[i]
                    tsl = slice(512 * n, 512 * (n + 1))
                    raw = raws.pop(i)
                    prot = qps.tile([128, 512], f32, tag="qps",
                                    name=f"prot{i}")
                    nc.tensor.matmul(prot[:], rT[:], raw[:], start=True,
                                     stop=True)
                    t1 = pp.tile([128, 512], bf16, tag="t1", bufs=3,
                                 name=f"t1_{i}")
                    nc.vector.tensor_tensor(t1[:], raw[:], cosT[:, tsl], mult)
                    t2 = pp.tile([128, 512], bf16, tag="t2", bufs=3,
                                 name=f"t2_{i}")
                    nc.gpsimd.tensor_tensor(t2[:], prot[:], sinT[:, tsl], mult)
                    if m == 0:
                        nc.vector.tensor_tensor(q01[:, tsl], t1[:], t2[:], add)
                    elif m == 1:
                        nc.vector.tensor_tensor(k01[:, tsl], t1[:], t2[:], add)
                    else:
                        nc.vector.tensor_tensor(q2[:, tsl], t1[0:64, :],
                                                t2[0:64, :], add)
                        nc.vector.tensor_tensor(k2[:, tsl], t1[64:128, :],
                                                t2[64:128, :], add)

                # q01/k01 with rope lagging two chunks
                for i in range(8):
                    emit_raw(i)
                    if i >= 2:
                        emit_rope(i - 2)
                emit_rope(6)
                emit_rope(7)

                # V projection, 3-term hi+lo fp8 DoubleRow, natural [t, d]
                for t in range(TQ):
                    tsl = slice(128 * t, 128 * (t + 1))
                    pv = qps.tile([128, 192], f32, tag="qps", name=f"pv{t}")
                    terms = [(xh, wA), (xl, wA), (xh, wAl)]
                    nmm = 0
                    for xi, wi in terms:
                        for p in range(3):
                            wap = (wi[:, p, :, 384:576] if wi is wA
                                   else wi[:, p, :, :])
                            nc.tensor.matmul(pv[:], xi[:, 2 * p:2 * p + 2, tsl],
                                             wap, start=(nmm == 0),
                                             stop=(nmm == 8), perf_mode=DR)
                            nmm += 1
                    nc.gpsimd.tensor_copy(
                        v_sb[:, t, :, 0:64],
                        pv[:].rearrange("p (h d) -> p h d", d=64))

                # q2k2 last, rope lagging one chunk
                for i in range(8, 12):
                    emit_raw(i)
                    if i > 8:
                        emit_rope(i - 1)
                emit_rope(11)

                qkv_ps_pool.__exit__(None, None, None)

                # outproj psum ring; reuses the qkv banks after it closes
                pout_ps_pool = tc.tile_pool(name=f"poutps{rep}", bufs=2,
                                            space="PSUM")
                sps = pout_ps_pool.__enter__()

                # ========== attention + out projection ==========
                qv = [(q01, 0), (q01, 64), (q2, 0)]
                kv = [(k01, 0), (k01, 64), (k2, 0)]

                for g in range(NGRP):
                    ana2 = {}      # qbl -> [128, 128] staging for heads 0+1
                    for h in range(HPG):
                        qt, qo = qv[h]
                        kt, ko = kv[h]
                        qT = qt[qo:qo + 64, :]
                        kT = kt[ko:ko + 64, :]
                        nj = 8 * (g + 1)
                        # pt for the whole (g, h): [128, 16, GW] bf16
                        ptb = pp.tile([128, 16, GW], bf16, tag="pt", bufs=2,
                                      name=f"pt{g}_{h}")
                        for j in range(nj):
                            dj = j - 8 * g
                            col0 = 128 * dj if dj >= 0 else 0
                            pscr = aps.tile([128, GW], f32, tag="pscr",
                                            name=f"pscr{g}_{h}_{j}")
                            for s0 in range(col0 - col0 % 512, GW, 512):
                                a0 = max(s0, col0)
                                nc.tensor.matmul(
                                    pscr[:, a0:s0 + 512],
                                    kT[:, 128 * j:128 * (j + 1)],
                                    qT[:, GW * g + a0:GW * g + s0 + 512],
                                    start=True, stop=True)
                            nc.scalar.activation(ptb[:, j, col0:],
                                                 pscr[:, col0:], Exp,
                                                 scale=SC)
                            if dj >= 0:
                                nc.gpsimd.tensor_tensor(
                                    ptb[:, j, col0:col0 + 128],
                                    ptb[:, j, col0:col0 + 128],
                                    maskA[:], mult)
                        # P@V natural: out[qb 128, 65] = sum_j pt_j^T @ v_j
                        for qq in range(2):       # po tiles of 4 q-blocks
                            po = ops.tile([128, 4, 65], f32, tag="po",
                                          name=f"po{g}_{h}_{qq}")
                            for qs in range(4):
                                qbl = 4 * qq + qs          # local q-block
                                njq = 8 * g + qbl + 1      # causal k-chunks
                                for j in range(njq):
                                    nc.tensor.matmul(
                                        po[:, qs, :],
                                        ptb[:, j, 128 * qbl:128 * (qbl + 1)],
                                        v_sb[:, j, h, :],
                                        start=(j == 0), stop=(j == njq - 1),
                                        skip_group_check=True)
                                # normalize + write natural attn (bf16);
                                # heads 0+1 share a [128,128] staging tile
                                # so the xbar transpose sees 128 free cols
                                tglob = 8 * g + qbl
                                tsl = slice(128 * tglob, 128 * (tglob + 1))
                                if h < 2:
                                    if h == 0:
                                        ana2[qbl] = pp.tile(
                                            [128, 128], bf16, tag="ana2",
                                            bufs=10, name=f"ana2_{g}_{qbl}")
                                    ana = ana2[qbl]
                                    nc.gpsimd.tensor_scalar(
                                        ana[:, 64 * h:64 * (h + 1)],
                                        po[:, qs, 0:64],
                                        po[:, qs, 64:65], None, divide)
                                    if h == 1:
                                        dq[0].dma_start_transpose(
                                            attnT_A[:, tsl], ana[:])
                                else:
                                    anb = pp.tile([128, 128], bf16,
                                                  tag="anaB", bufs=4,
                                                  name=f"anaB{g}_{qbl}")
                                    nc.gpsimd.memset(anb[:, 64:128], 0.0)
                                    nc.gpsimd.tensor_scalar(
                                        anb[:, 0:64], po[:, qs, 0:64],
                                        po[:, qs, 64:65], None, divide)
                                    dq[0].dma_start_transpose(
                                        attnT_B[:, tsl], anb[:])

                    # out projection for this g's t-range (bf16)
                    for t in range((TQ // NGRP) * g, (TQ // NGRP) * (g + 1)):
                        tsl = slice(128 * t, 128 * (t + 1))
                        for c0, cn in ((0, 512), (512, 256)):
                            pout = sps.tile([128, cn], f32, tag="pout",
                                            name=f"pout{t}_{c0}")
                            nc.tensor.matmul(pout[:], attnT_A[:, tsl],
                                             woA[:, c0:c0 + cn], start=True,
                                             stop=False)
                            nc.tensor.matmul(pout[:], attnT_B[0:64, tsl],
                                             woB[:, c0:c0 + cn], start=False,
                                             stop=True)
                            osb = pp.tile([128, cn], bf16, tag=f"osb{c0}",
                                          bufs=3, name=f"osb{t}_{c0}")
                            nc.vector.tensor_copy(osb[:], pout[:])
                            qi = 0 if (t + (1 if c0 else 0)) % 3 else 1
                            dq[qi].dma_start(out_d[tsl, c0:c0 + cn], osb[:])

                pout_ps_pool.__exit__(None, None, None)
                po_ps_pool.__exit__(None, None, None)
                attn_ps_pool.__exit__(None, None, None)

    nc.compile()
    return nc


def _host_inputs(x, w_qkv, w_out):
    """Build the 8 per-core input maps."""
    import ml_dtypes
    f8 = ml_dtypes.float8_e4m3
    bf = ml_dtypes.bfloat16

    inv_freq = 1.0 / (ROPE_BASE ** (np.arange(0, D, 2, dtype=np.float32) / D))
    t = np.arange(T, dtype=np.float32)
    freqs = t[:, None] * inv_freq[None, :]          # [T, D/2]
    emb = np.concatenate([freqs, freqs], axis=-1)   # [T, D]
    cosT = np.ascontiguousarray(np.cos(emb).T.astype(np.float32))  # [D, T]
    sinT = np.ascontiguousarray(np.sin(emb).T.astype(np.float32))
    cosT2 = np.concatenate([cosT, cosT], axis=0).astype(bf)  # [128, T]
    sinT2 = np.concatenate([sinT, sinT], axis=0).astype(bf)

    R = np.zeros((D, D), np.float32)
    R[0:32, 32:64] = -np.eye(32)
    R[32:64, 0:32] = np.eye(32)
    R2 = np.zeros((128, 128), np.float32)
    R2[0:64, 0:64] = R
    R2[64:128, 64:128] = R
    rT = np.ascontiguousarray(R2.T).astype(bf)

    maskA = np.zeros((128, 128), np.float32)
    for kr in range(128):
        maskA[kr, kr:] = 1.0
    maskA = maskA.astype(bf)

    wq = w_qkv[0:C]
    wk = w_qkv[C:2 * C]
    wv = w_qkv[2 * C:3 * C]

    maps = []
    for core in range(NG):
        b, hg = core // 4, core % 4
        base = HPG * D * hg
        Wrows = np.concatenate([
            wq[base:base + 128],
            wk[base:base + 128],
            wq[base + 128:base + 192],
            wk[base + 128:base + 192],
            wv[base:base + 192],
        ], axis=0).astype(np.float32) * SW               # [576, C]
        # wA8[p, pair, half, col] = Wrows[col, 128*(2*pair+half)+p]
        WT = np.ascontiguousarray(
            Wrows.T.reshape(3, 2, 128, 576).transpose(2, 0, 1, 3))
        wA8 = WT.astype(f8)
        # lo residual only needed for the v columns (384:576)
        wAl8 = np.ascontiguousarray(
            (WT - wA8.astype(np.float32))[:, :, :, 384:576]).astype(f8)
        woA = np.ascontiguousarray(
            w_out[:, base:base + 128].T).astype(bf)      # [128, C]
        woB = np.ascontiguousarray(
            w_out[:, base + 128:base + 192].T).astype(bf)  # [64, C]
        # xh8/xl8[p, cb, t] = x[b][t, 128*cb+p] hi/lo fp8 split
        xT = np.ascontiguousarray(
            x[b].T.reshape(6, 128, T).transpose(1, 0, 2)).astype(np.float32)
        xh8 = xT.astype(f8)
        xl8 = (xT - xh8.astype(np.float32)).astype(f8)
        maps.append({
            "xh8": xh8, "xl8": xl8, "wA8": wA8, "wAl8": wAl8,
            "woA": woA, "woB": woB,
            "cosT": cosT2, "sinT": sinT2,
            "rT": rT, "maskA": maskA,
        })
    return maps


def kernel(x, w_qkv, w_out):
    from concourse.bass_utils import run_bass_kernel_spmd

    if "nc" not in _CACHE:
        _CACHE["nc"] = _build_nc()
    nc = _CACHE["nc"]

    maps = _host_inputs(np.asarray(x, np.float32),
                        np.asarray(w_qkv, np.float32),
                        np.asarray(w_out, np.float32))
    res = run_bass_kernel_spmd(nc, maps, core_ids=list(range(NG))).results
    parts = np.stack([np.asarray(r["out"], np.float32) for r in res])
    out = np.zeros((B, T, C), np.float32)
    for b in range(B):
        out[b] = parts[4 * b:4 * (b + 1)].sum(axis=0)
    return out
